# revision 3
# baseline (speedup 1.0000x reference)
"""ContactMapHead Trainium2 kernel (v10: fp8 DoubleRow projection).

Reference computation (per batch b):
    h = relu(X @ W^T + pb)            # [S, DP]
    scores = (h @ h^T) * cw + cb      # [S, S]  -- symmetric!

Sharding over 8 NeuronCores: core c handles batch b = c//2 with roll
offset off = (c%2)*1024 applied to X on the host. Each core computes
hT = relu(W @ XT + pb) for its full (rolled) batch, then emits the
circulant band of the symmetric score map: local tile rows i_t in 0..7
(tiles of 128), local cols j_t in i_t..i_t+8 (9 tiles of 128). Across
the two cores of a batch pair plus host-side transpose mirroring this
covers all 16x16 global tiles exactly.

v10 design changes over v9 (48.4us measured):
- Projection in fp8e4 with MatmulPerfMode.DoubleRow: the slot dim
  carries (Xhi, Xlo) - a two-term fp8 expansion of X (exact to ~2^-8
  rel) - against W duplicated across slots. Each DR matmul contracts
  256 virtual rows at 0.5 cycles/column: the projection drops from
  8 to 8 matmuls of half cost (with LDW-share pairs, see below).
  Host folds a x64 power-of-2 scale into Wq and compensates via
  pb*64 and cw/4096 (relu is positively homogeneous).
- W error stays (~1.3e-2 sim'd vs 2e-2 tol); X expansion kills the
  X quantization error. h stays bf16; the band matmuls stay bf16.
- Input DRAM layout [P, NCH, KT, 2, CHK] makes each chunk 8KB
  contiguous per partition (vs 512B lines in v9): DMA descriptors go
  from 512B to 8KB, lifting stream rate from ~280 to ring cap. Each
  chunk is split across BOTH HWDGE rings by partition halves.
- Proj emits per (k, pt) two consecutive 256-wide matmuls with the
  SAME stationary operand so the legalizer may elide the second
  LDWEIGHTS (LDW otherwise bounds narrow fp8 matmuls).
"""

import numpy as np
import ml_dtypes

from concourse import bacc, masks, mybir, tile

BF = ml_dtypes.bfloat16
F8 = ml_dtypes.float8_e4m3

P = 128
B, S, D = 4, 2048, 1024
DP = 256  # projection dim
NCORES = 8
KT = D // P  # 8 k-tiles over D
PT = DP // P  # 2 p-tiles over DP
CHK = 512  # input DMA chunk width (s columns)
NCH = S // CHK  # 4 chunks
NROW = 8  # local band rows (tiles of 128) per core
BANDW = 9 * P  # 1152 band columns per row
SEG = BANDW // 3  # 384-col band chunks
NWARM = 9
SW = 64.0  # power-of-2 scale folded into Wq (relu-homogeneous)

f32 = mybir.dt.float32
bf16 = mybir.dt.bfloat16
fp8 = mybir.dt.float8e4
DR = mybir.MatmulPerfMode.DoubleRow


def _build_nc():
    nc = bacc.Bacc()
    xt = nc.declare_dram_parameter("xt", [P, NCH, KT, 2, CHK], fp8, isOutput=False)
    wt = nc.declare_dram_parameter("wt", [P, KT, 2, DP], fp8, isOutput=False)
    pb = nc.declare_dram_parameter("pb", [DP], f32, isOutput=False)
    cwb = nc.declare_dram_parameter("cwb", [2], f32, isOutput=False)
    out = nc.declare_dram_parameter("out", [NROW, P, BANDW], bf16, isOutput=True)

    with tile.TileContext(nc) as tc:
        _body(nc, tc, xt, wt, pb, cwb, out)
    nc.compile()
    return nc


def _body(nc, tc, xt, wt, pb, cwb, out):
    mult = mybir.AluOpType.mult
    add = mybir.AluOpType.add
    Relu = mybir.ActivationFunctionType.Relu
    Ident = mybir.ActivationFunctionType.Identity

    with (
        tc.tile_pool(name="const", bufs=1) as cpool,
        tc.tile_pool(name="orow", bufs=NROW) as opool,
        tc.tile_pool(name="pj", bufs=4, space="PSUM") as pj,
        tc.tile_pool(name="pw", bufs=4, space="PSUM") as pw,
    ):
        # ---- PE warm-up: f32 matmuls on an on-chip identity (no DMA
        # dependency). They run while the input streams in and trip the
        # HAM activity monitor so real work starts at 2.4 GHz.
        ident = cpool.tile([P, P], f32, tag="ident")
        masks.make_identity(nc, ident[:])
        wps = pw.tile([P, SEG], f32, tag="pw", name="warm")
        for _ in range(NWARM):
            nc.tensor.matmul(wps[:, 0:P], ident[:], ident[:], start=True, stop=True)

        # ---- wt + constants on the gpsimd SWDGE ring, keeping the two
        # HWDGE rings clear for x chunks.
        wt_t = cpool.tile([P, KT, 2, DP], fp8, tag="wt_t")
        nc.gpsimd.dma_start(wt_t[:], wt.ap()[:])

        pb_t = cpool.tile([P, PT], f32, tag="pb_t")
        nc.gpsimd.dma_start(pb_t[:], pb.ap().rearrange("(t p) -> p t", p=P))

        cwb_t = cpool.tile([P, 2], f32, tag="cwb_t")
        nc.gpsimd.dma_start(cwb_t[:], cwb.ap().partition_broadcast(P))

        # ---- x chunks: each 1MB chunk (8KB contiguous per partition)
        # is split across BOTH HWDGE rings by partition halves so the
        # two rings stream symmetrically and a chunk completes in ~1.6us.
        xtile = cpool.tile([P, NCH, KT, 2, CHK], fp8, tag="xtile")
        xv = xt.ap()
        for ch in range(NCH):
            nc.sync.dma_start(xtile[0:64, ch], xv[0:64, ch])
            nc.scalar.dma_start(xtile[64:128, ch], xv[64:128, ch])

        # hT for the whole local map; relu writes per (pt, chunk) slices
        ht = cpool.tile([P, PT, S], bf16, tag="ht")

        def project(ch):
            c0 = ch * CHK
            for pt in range(PT):
                pjs = pj.tile([P, CHK], f32, tag="pj", name="pj")
                for k in range(KT):
                    wv = wt_t[:, k, :, pt * P : (pt + 1) * P]
                    # two half-width matmuls with the SAME stationary
                    # tensor back to back (LDW elision opportunity).
                    # start=True zeroes the WHOLE 2KB psum bank, so only
                    # the very first matmul into the bank carries it.
                    nc.tensor.matmul(
                        pjs[:, 0:256],
                        wv,
                        xtile[:, ch, k, :, 0:256],
                        start=(k == 0),
                        stop=(k == KT - 1),
                        perf_mode=DR,
                    )
                    nc.tensor.matmul(
                        pjs[:, 256:512],
                        wv,
                        xtile[:, ch, k, :, 256:512],
                        start=False,
                        stop=(k == KT - 1),
                        perf_mode=DR,
                        skip_group_check=True,
                    )
                # psum holds 64*(X@W^T); relu(psum + 64*pb) = 64*h.
                # pt0 -> ScalarE, pt1 -> VectorE (parallel PSUM banks).
                if pt == 0:
                    nc.scalar.activation(
                        ht[:, pt, c0 : c0 + CHK],
                        pjs[:],
                        Relu,
                        bias=pb_t[:, pt : pt + 1],
                    )
                else:
                    nc.vector.tensor_scalar(
                        ht[:, pt, c0 : c0 + CHK],
                        pjs[:],
                        pb_t[:, pt : pt + 1],
                        0.0,
                        add,
                        mybir.AluOpType.max,
                    )

        def emit_pair_row(i_t):
            """Band row i_t: out[i_t] = cw' * hT_i^T @ hT[band cols] + cb."""
            base = i_t * P
            psums = []
            for pt in range(PT):
                for si in range(3):
                    if pt == 0:
                        psums.append(pw.tile([P, SEG], f32, tag="pw", name="pw"))
                    c0 = base + si * SEG
                    nc.tensor.matmul(
                        psums[si][:],
                        ht[:, pt, base : base + P],
                        ht[:, pt, c0 : c0 + SEG],
                        start=(pt == 0),
                        stop=(pt == PT - 1),
                    )
            orow = opool.tile([P, BANDW], bf16, tag="orow", name="orow")
            tail = i_t >= NROW - 2
            for si in range(3):
                dst = orow[:, si * SEG : (si + 1) * SEG]
                if (i_t * 3 + si) % 2 == 0:
                    nc.vector.tensor_scalar(
                        dst, psums[si][:], cwb_t[:, 0:1], cwb_t[:, 1:2], mult, add
                    )
                else:
                    nc.scalar.activation(
                        dst, psums[si][:], Ident,
                        bias=cwb_t[:, 1:2], scale=cwb_t[:, 0:1],
                    )
                if tail:
                    # last rows: drain per segment so the final DMA is small
                    eng = nc.sync if (i_t + si) % 2 == 0 else nc.gpsimd
                    eng.dma_start(
                        out.ap()[i_t][:, si * SEG : (si + 1) * SEG], dst
                    )
            if not tail:
                eng = nc.sync if i_t % 2 == 0 else nc.gpsimd
                eng.dma_start(out.ap()[i_t], orow[:])

        # dovetail: rows i need h cols < i*128+1152; chunks are 512 wide
        # so rows 0-3 unlock after chunk 2, rows 4-7 after chunk 3.
        project(0)
        project(1)
        project(2)
        for i_t in range(4):
            emit_pair_row(i_t)
        project(3)
        for i_t in range(4, NROW):
            emit_pair_row(i_t)


_NC_CACHE = None


def _get_nc():
    global _NC_CACHE
    if _NC_CACHE is None:
        _NC_CACHE = _build_nc()
    return _NC_CACHE


def _pack_x(XT_hi, XT_lo):
    """two [D, S] fp8 views -> [P, NCH, KT, 2, CHK] with d = k*P + p."""
    # [D, S] -> [KT, P, NCH, CHK] -> stack slot -> [P, NCH, KT, 2, CHK]
    def r(a):
        return a.reshape(KT, P, NCH, CHK)

    st = np.stack([r(XT_hi), r(XT_lo)], axis=3)  # [KT, P, NCH, 2, CHK]
    return np.ascontiguousarray(st.transpose(1, 2, 0, 3, 4))


def _make_in_maps(hidden_states, proj_w, proj_b, clf_w, clf_b):
    hs = np.asarray(hidden_states, dtype=np.float32)
    wv = np.asarray(proj_w, dtype=np.float32)
    pbv = np.ascontiguousarray(
        np.asarray(proj_b, dtype=np.float32).reshape(DP) * SW
    )
    cw = float(np.asarray(clf_w).reshape(-1)[0])
    cb = float(np.asarray(clf_b).reshape(-1)[0])
    cwbv = np.array([cw / (SW * SW), cb], dtype=np.float32)

    # W: [DP, D] scaled by SW, fp8, duplicated across the slot dim
    wq = (wv * SW).astype(F8)  # [DP, D]
    wtv = np.empty((P, KT, 2, DP), dtype=F8)
    wr = np.ascontiguousarray(wq.T).reshape(KT, P, DP)  # [k, p, m]
    wtv[:, :, 0, :] = wr.transpose(1, 0, 2)
    wtv[:, :, 1, :] = wr.transpose(1, 0, 2)

    in_maps = []
    for b in range(B):
        xT = np.ascontiguousarray(hs[b].T)  # [D, S] f32
        for half in range(2):
            xr = np.roll(xT, -S // 2, axis=1) if half else xT
            xhi = xr.astype(F8)
            xlo = (xr - xhi.astype(np.float32)).astype(F8)
            in_maps.append(
                {
                    "xt": _pack_x(xhi, xlo),
                    "wt": wtv,
                    "pb": pbv,
                    "cwb": cwbv,
                }
            )
    return in_maps


def _assemble(results):
    scores = np.empty((B, S, S), np.float32)
    for c in range(NCORES):
        b, half = divmod(c, 2)
        o = np.asarray(results[c]["out"], dtype=np.float32)  # [NROW, P, BANDW]
        for i_t in range(NROW):
            gi = i_t + NROW * half
            strip = o[i_t]
            for lj in range(i_t, i_t + 9):
                gj = (lj + NROW * half) % 16
                V = strip[:, (lj - i_t) * P : (lj - i_t + 1) * P]
                scores[b, gi * P : (gi + 1) * P, gj * P : (gj + 1) * P] = V
                if gj != gi:
                    scores[b, gj * P : (gj + 1) * P, gi * P : (gi + 1) * P] = V.T
    return scores


def kernel(hidden_states, proj_w, proj_b, clf_w, clf_b):
    from concourse.bass_utils import run_bass_kernel_spmd

    nc = _get_nc()
    in_maps = _make_in_maps(hidden_states, proj_w, proj_b, clf_w, clf_b)
    res = run_bass_kernel_spmd(nc, in_maps, core_ids=list(range(NCORES)))
    return _assemble(res.results)


def run_traced(hidden_states, proj_w, proj_b, clf_w, clf_b):
    """Like kernel(), but also returns BassKernelResults with trace info."""
    from concourse.bass_utils import run_bass_kernel_spmd

    nc = _get_nc()
    in_maps = _make_in_maps(hidden_states, proj_w, proj_b, clf_w, clf_b)
    res = run_bass_kernel_spmd(
        nc, in_maps, core_ids=list(range(NCORES)), trace=True
    )
    return _assemble(res.results), res
